# revision 1
# baseline (speedup 1.0000x reference)
"""Criss-cross attention (width=1) Trainium2 Bass kernel.

Math note: for width=1 the criss-cross module collapses to plain unmasked
softmax attention.  The diagonal of energy_H is masked to -inf, but the
"width" logit energy_W[i] equals that same diagonal value q_i.k_i, and it is
re-appended as the (n+1)-th softmax entry.  So per query i the softmax runs
over exactly {q_i.k_j : j=0..n-1}, and

    out = gamma * (V @ softmax_j(Q^T K)) + x
    Q = relu(bn1(w_q x)),  K = relu(bn2(w_k x)),  V = relu(bn3(w_v x))

Sharding: 8 cores = (4 batches) x (2 query halves).  Each core computes
K, V for all 4096 keys of its batch and attention output for its 2048
queries.  Zero cross-core communication.

On-chip layout per core (all matmuls bf16 in / f32 psum accumulate):
  S^T[j,i] = K^T Q tiles [j=128, i=512]   (keys on partitions)
  E = exp(S^T) -> bf16 (no max subtraction needed: logits <= ~30)
  O^T[i, 0:256] += E_tile^T @ VT[j, 0:256]; column 256 of rhs is ones so
  O^T[i, 256] accumulates Z_i = sum_j exp.  Epilogue: per-partition
  (per-query) reciprocal, scale, PE-transpose back to [c, i], add residual.
"""

import os
import numpy as np
import ml_dtypes

_B, _C, _N, _CR = 4, 256, 4096, 32
_NCORES = 8
_HALF = _N // 2  # queries per core
_EPS = 1e-5

_BUILD_CACHE: dict = {}


def _build(has_bq: bool, has_bk: bool, has_bv: bool):
    import concourse.mybir as mybir
    import concourse.tile as tile
    from concourse import bacc

    f32 = mybir.dt.float32
    bf16 = mybir.dt.bfloat16
    AF = mybir.ActivationFunctionType
    ALU = mybir.AluOpType

    nc = bacc.Bacc("TRN2", target_bir_lowering=False, debug=False)

    x_d = nc.dram_tensor("x", [_C, _N], f32, kind="ExternalInput")
    xq_d = nc.dram_tensor("xq", [_C, _HALF], f32, kind="ExternalInput")
    wq_d = nc.dram_tensor("wqt", [_C, _CR], bf16, kind="ExternalInput")
    wk_d = nc.dram_tensor("wkt", [_C, _CR], bf16, kind="ExternalInput")
    wv_d = nc.dram_tensor("wvt", [_C, _C], bf16, kind="ExternalInput")
    g_d = nc.dram_tensor("gvec", [128, 1], f32, kind="ExternalInput")
    bq_d = nc.dram_tensor("bq", [_CR, 1], f32, kind="ExternalInput") if has_bq else None
    bk_d = nc.dram_tensor("bk", [_CR, 1], f32, kind="ExternalInput") if has_bk else None
    bv_d = nc.dram_tensor("bv", [1, _C], bf16, kind="ExternalInput") if has_bv else None
    out_d = nc.dram_tensor("out", [_C, _HALF], f32, kind="ExternalOutput")

    NJ = _N // 128        # 32 key blocks
    NI5 = _HALF // 512    # 4 query super-blocks
    VTW = _C + 1          # 257: V^T columns + ones column for Z

    with tile.TileContext(nc) as tc:
        with tc.tile_pool(name="persist", bufs=1) as pers, \
             tc.tile_pool(name="work", bufs=2) as work:
            # ---- persistent SBUF tensors ----
            ident = pers.tile([128, 128], f32, name="ident")
            from concourse.masks import make_identity
            make_identity(nc, ident)

            g_sb = pers.tile([128, 1], f32, name="g_sb")
            nc.sync.dma_start(g_sb, g_d.ap())

            wq_sb = pers.tile([128, 2 * _CR], bf16, name="wq_sb")
            nc.sync.dma_start(wq_sb[:, 0:_CR], wq_d.ap()[0:128, :])
            nc.sync.dma_start(wq_sb[:, _CR:2 * _CR], wq_d.ap()[128:256, :])
            wk_sb = pers.tile([128, 2 * _CR], bf16, name="wk_sb")
            nc.sync.dma_start(wk_sb[:, 0:_CR], wk_d.ap()[0:128, :])
            nc.sync.dma_start(wk_sb[:, _CR:2 * _CR], wk_d.ap()[128:256, :])
            wv_sb = pers.tile([128, 2 * _C], bf16, name="wv_sb")
            nc.sync.dma_start(wv_sb[:, 0:_C], wv_d.ap()[0:128, :])
            nc.sync.dma_start(wv_sb[:, _C:2 * _C], wv_d.ap()[128:256, :])

            if has_bq:
                bq_sb = pers.tile([_CR, 1], f32, name="bq_sb")
                nc.sync.dma_start(bq_sb, bq_d.ap())
            if has_bk:
                bk_sb = pers.tile([_CR, 1], f32, name="bk_sb")
                nc.sync.dma_start(bk_sb, bk_d.ap())
            if has_bv:
                bv_sb = pers.tile([1, _C], bf16, name="bv_sb")
                nc.sync.dma_start(bv_sb, bv_d.ap())
                ones_row = pers.tile([1, 128], bf16, name="ones_row")
                nc.any.memset(ones_row, 1.0)

            xbf0 = pers.tile([128, _N], bf16, name="xbf0")
            xbf1 = pers.tile([128, _N], bf16, name="xbf1")
            xq0 = pers.tile([128, _HALF], f32, name="xq0")
            xq1 = pers.tile([128, _HALF], f32, name="xq1")
            xqbf0 = pers.tile([128, _HALF], bf16, name="xqbf0")
            xqbf1 = pers.tile([128, _HALF], bf16, name="xqbf1")
            k_sb = pers.tile([_CR, _N], bf16, name="k_sb")
            q_sb = pers.tile([_CR, _HALF], bf16, name="q_sb")
            vt_sb = pers.tile([128, NJ * VTW], bf16, name="vt_sb")

            # ---- phase 0: load x, cast, compute K / Q / V^T ----
            CH = 1024
            with tc.tile_pool(name="prep_ps", space="PSUM", bufs=2) as pps:
                for ch in range(_N // CH):
                    sl = slice(ch * CH, (ch + 1) * CH)
                    xf0 = work.tile([128, CH], f32, name="xf0", tag="xf", bufs=3)
                    nc.sync.dma_start(xf0, x_d.ap()[0:128, sl])
                    nc.vector.tensor_copy(xbf0[:, sl], xf0)
                    xf1 = work.tile([128, CH], f32, name="xf1", tag="xf", bufs=3)
                    nc.sync.dma_start(xf1, x_d.ap()[128:256, sl])
                    nc.vector.tensor_copy(xbf1[:, sl], xf1)
                for ch in range(_HALF // CH):
                    sl = slice(ch * CH, (ch + 1) * CH)
                    nc.sync.dma_start(xq0[:, sl], xq_d.ap()[0:128, sl])
                    nc.vector.tensor_copy(xqbf0[:, sl], xq0[:, sl])
                    nc.sync.dma_start(xq1[:, sl], xq_d.ap()[128:256, sl])
                    nc.vector.tensor_copy(xqbf1[:, sl], xq1[:, sl])

                # K: [32, n]
                for b5 in range(_N // 512):
                    sl = slice(b5 * 512, (b5 + 1) * 512)
                    kp = pps.tile([_CR, 512], f32, name="kp", tag="prep", bufs=2)
                    nc.tensor.matmul(kp, wk_sb[:, 0:_CR], xbf0[:, sl], start=True, stop=False)
                    nc.tensor.matmul(kp, wk_sb[:, _CR:2 * _CR], xbf1[:, sl], start=False, stop=True)
                    if has_bk:
                        nc.vector.tensor_scalar(k_sb[:, sl], kp, bk_sb, 0.0, ALU.add, ALU.max)
                    else:
                        nc.vector.tensor_scalar_max(k_sb[:, sl], kp, 0.0)
                # Q: [32, half]
                for b5 in range(_HALF // 512):
                    sl = slice(b5 * 512, (b5 + 1) * 512)
                    qp = pps.tile([_CR, 512], f32, name="qp", tag="prep", bufs=2)
                    nc.tensor.matmul(qp, wq_sb[:, 0:_CR], xqbf0[:, sl], start=True, stop=False)
                    nc.tensor.matmul(qp, wq_sb[:, _CR:2 * _CR], xqbf1[:, sl], start=False, stop=True)
                    if has_bq:
                        nc.vector.tensor_scalar(q_sb[:, sl], qp, bq_sb, 0.0, ALU.add, ALU.max)
                    else:
                        nc.vector.tensor_scalar_max(q_sb[:, sl], qp, 0.0)
                # V^T: per key block [j=128, c=256], relu then * gamma, ones col
                for jb in range(NJ):
                    jsl = slice(jb * 128, (jb + 1) * 128)
                    vp = pps.tile([128, _C], f32, name="vp", tag="prep", bufs=2)
                    nc.tensor.matmul(vp, xbf0[:, jsl], wv_sb[:, 0:_C], start=True,
                                     stop=not has_bv)
                    nc.tensor.matmul(vp, xbf1[:, jsl], wv_sb[:, _C:2 * _C], start=False,
                                     stop=not has_bv)
                    if has_bv:
                        nc.tensor.matmul(vp, ones_row, bv_sb, start=False, stop=True)
                    vsl = slice(jb * VTW, jb * VTW + _C)
                    nc.vector.tensor_scalar(vt_sb[:, vsl], vp, 0.0, g_sb, ALU.max, ALU.mult)
                    nc.any.memset(vt_sb[:, jb * VTW + _C: (jb + 1) * VTW], 1.0)

            # ---- phase 1: attention ----
            with tc.tile_pool(name="att_ps", space="PSUM", bufs=1) as aps:
                for i5 in range(NI5):
                    isl = slice(i5 * 512, (i5 + 1) * 512)
                    ots = [
                        aps.tile([128, VTW], f32, name=f"ot{s}", tag=f"ot{s}", bufs=1)
                        for s in range(4)
                    ]
                    for jb in range(NJ):
                        st = aps.tile([128, 512], f32, name="st", tag="st", bufs=2)
                        nc.tensor.matmul(st, k_sb[:, jb * 128:(jb + 1) * 128],
                                         q_sb[:, isl], start=True, stop=True)
                        e_sb = work.tile([128, 512], bf16, name="e_sb", tag="e", bufs=3)
                        nc.scalar.activation(e_sb, st, AF.Exp)
                        for s in range(4):
                            nc.tensor.matmul(
                                ots[s],
                                e_sb[:, s * 128:(s + 1) * 128],
                                vt_sb[:, jb * VTW:(jb + 1) * VTW],
                                start=(jb == 0), stop=(jb == NJ - 1),
                            )
                    for s in range(4):
                        i0 = i5 * 512 + s * 128
                        rz = work.tile([128, 1], f32, name="rz", tag="rz", bufs=2)
                        nc.vector.reciprocal(rz, ots[s][:, _C:_C + 1])
                        onrm = work.tile([128, _C], f32, name="onrm", tag="onrm", bufs=2)
                        nc.vector.tensor_scalar_mul(onrm, ots[s][:, 0:_C], rz)
                        for chh in range(2):
                            tp = aps.tile([128, 128], f32, name="tp", tag="tp", bufs=2)
                            nc.tensor.transpose(tp, onrm[:, chh * 128:(chh + 1) * 128], ident)
                            res = work.tile([128, 128], f32, name="res", tag="res", bufs=3)
                            xq_t = xq0 if chh == 0 else xq1
                            nc.vector.tensor_add(res, tp, xq_t[:, i0:i0 + 128])
                            nc.sync.dma_start(
                                out_d.ap()[chh * 128:(chh + 1) * 128, i0:i0 + 128], res)

    nc.compile()
    return nc


def _get_nc(has_bq, has_bk, has_bv):
    key = (has_bq, has_bk, has_bv)
    if key not in _BUILD_CACHE:
        _BUILD_CACHE[key] = _build(*key)
    return _BUILD_CACHE[key]


def kernel(x, w_q, w_k, w_v,
           bn1_scale, bn1_bias, bn1_mean, bn1_var,
           bn2_scale, bn2_bias, bn2_mean, bn2_var,
           bn3_scale, bn3_bias, bn3_mean, bn3_var,
           gamma, _trace=False):
    from concourse.bass_utils import run_bass_kernel_spmd

    x = np.asarray(x, dtype=np.float32)
    gamma_f = float(np.asarray(gamma).reshape(-1)[0])
    bf = ml_dtypes.bfloat16

    def fold(w, s, b, m, v):
        a = np.asarray(s, np.float32) / np.sqrt(np.asarray(v, np.float32) + _EPS)
        return (np.asarray(w, np.float32) * a[:, None],
                np.asarray(b, np.float32) - np.asarray(m, np.float32) * a)

    wqf, bq = fold(w_q, bn1_scale, bn1_bias, bn1_mean, bn1_var)
    wkf, bk = fold(w_k, bn2_scale, bn2_bias, bn2_mean, bn2_var)
    wvf, bv = fold(w_v, bn3_scale, bn3_bias, bn3_mean, bn3_var)
    has_bq = bool(np.any(bq != 0.0))
    has_bk = bool(np.any(bk != 0.0))
    has_bv = bool(np.any(bv != 0.0))

    nc = _get_nc(has_bq, has_bk, has_bv)

    wqt = np.ascontiguousarray(wqf.T).astype(bf)          # [c_in, cr]
    wkt = np.ascontiguousarray(wkf.T).astype(bf)
    wvt = np.ascontiguousarray(wvf.T).astype(bf)          # [c_in, c_out]
    gvec = np.full((128, 1), gamma_f, dtype=np.float32)

    in_maps = []
    for core in range(_NCORES):
        b, h = divmod(core, 2)
        m = {
            "x": np.ascontiguousarray(x[b]),
            "xq": np.ascontiguousarray(x[b][:, h * _HALF:(h + 1) * _HALF]),
            "wqt": wqt, "wkt": wkt, "wvt": wvt, "gvec": gvec,
        }
        if has_bq:
            m["bq"] = np.ascontiguousarray(bq.reshape(_CR, 1))
        if has_bk:
            m["bk"] = np.ascontiguousarray(bk.reshape(_CR, 1))
        if has_bv:
            m["bv"] = np.ascontiguousarray(bv.reshape(1, _C)).astype(bf)
        in_maps.append(m)

    res = run_bass_kernel_spmd(nc, in_maps, core_ids=list(range(_NCORES)),
                               trace=_trace)

    out = np.empty((_B, _C, _N), dtype=np.float32)
    for core in range(_NCORES):
        b, h = divmod(core, 2)
        out[b, :, h * _HALF:(h + 1) * _HALF] = res.results[core]["out"]
    if _trace:
        kernel.last_results = res
    return out


# revision 2
# speedup vs baseline: 1.1654x; 1.1654x over previous
"""Criss-cross attention (width=1) Trainium2 Bass kernel.

Math note: for width=1 the criss-cross module collapses to plain unmasked
softmax attention.  The diagonal of energy_H is masked to -inf, but the
"width" logit energy_W[i] equals that same diagonal value q_i.k_i, and it is
re-appended as the (n+1)-th softmax entry.  So per query i the softmax runs
over exactly {q_i.k_j : j=0..n-1}, and

    out = gamma * (V @ softmax_j(Q^T K)) + x
    Q = relu(bn1(w_q x)),  K = relu(bn2(w_k x)),  V = relu(bn3(w_v x))

Sharding: 8 cores = (4 batches) x (2 query halves).  Each core computes
K, V for all 4096 keys of its batch and attention output for its 2048
queries.  Zero cross-core communication.

Per-core structure (matmuls bf16-in / f32-psum):
  S^T[j,i] = K^T Q, computed pair-packed: two K row-groups (K=32
  contraction each) run concurrently via tile_position, filling a
  [128, 1024] PSUM pair tile (two key blocks x 512 queries).
  E = exp(S^T) -> bf16 (no max subtraction needed: logits <= ~30).
  O^T[i, 0:256] accumulates E_blk^T @ [V^T | 1]; the ones column makes
  O^T[i, 256] = Z_i.  Epilogue: per-partition reciprocal, scale (gamma
  pre-folded into V^T), DMA-transpose back to [c, i], residual add.
  The j-loop is software-pipelined: PE runs PV of pair p-1 while ACT
  exponentiates pair p, keeping the tensor engine dense (HAM-warm).
"""

import os
import numpy as np
import ml_dtypes

_B, _C, _N, _CR = 4, 256, 4096, 32
_NCORES = 8
_HALF = _N // 2  # queries per core
_EPS = 1e-5

_BUILD_CACHE: dict = {}


def _build(has_bq: bool, has_bk: bool, has_bv: bool):
    import concourse.mybir as mybir
    import concourse.tile as tile
    from concourse import bacc

    f32 = mybir.dt.float32
    bf16 = mybir.dt.bfloat16
    AF = mybir.ActivationFunctionType
    ALU = mybir.AluOpType

    nc = bacc.Bacc("TRN2", target_bir_lowering=False, debug=False)

    x_d = nc.dram_tensor("x", [_C, _N], f32, kind="ExternalInput")
    xq_d = nc.dram_tensor("xq", [_C, _HALF], f32, kind="ExternalInput")
    wq_d = nc.dram_tensor("wqt2", [_C, 2 * _CR], bf16, kind="ExternalInput")
    wk_d = nc.dram_tensor("wkt2", [_C, 2 * _CR], bf16, kind="ExternalInput")
    wv_d = nc.dram_tensor("wvt", [_C, _C], bf16, kind="ExternalInput")
    g_d = nc.dram_tensor("gvec", [128, 1], f32, kind="ExternalInput")
    bq_d = nc.dram_tensor("bq2", [2 * _CR, 1], f32, kind="ExternalInput") if has_bq else None
    bk_d = nc.dram_tensor("bk2", [2 * _CR, 1], f32, kind="ExternalInput") if has_bk else None
    bv_d = nc.dram_tensor("bv", [1, _C], bf16, kind="ExternalInput") if has_bv else None
    out_d = nc.dram_tensor("out", [_C, _HALF], f32, kind="ExternalOutput")

    NJ = _N // 128        # 32 key blocks
    NP = NJ // 2          # 16 key pairs
    NI5 = _HALF // 512    # 4 query super-blocks
    VTW = _C + 1          # 257: V^T columns + ones column for Z

    with tile.TileContext(nc) as tc:
        with tc.tile_pool(name="persist", bufs=1) as pers, \
             tc.tile_pool(name="work", bufs=2) as work:
            # ---- persistent SBUF tensors ----
            g_sb = pers.tile([128, 1], f32, name="g_sb")
            nc.sync.dma_start(g_sb, g_d.ap())

            wq_sb = pers.tile([128, 4 * _CR], bf16, name="wq_sb")
            nc.sync.dma_start(wq_sb[:, 0:2 * _CR], wq_d.ap()[0:128, :])
            nc.sync.dma_start(wq_sb[:, 2 * _CR:4 * _CR], wq_d.ap()[128:256, :])
            wk_sb = pers.tile([128, 4 * _CR], bf16, name="wk_sb")
            nc.sync.dma_start(wk_sb[:, 0:2 * _CR], wk_d.ap()[0:128, :])
            nc.sync.dma_start(wk_sb[:, 2 * _CR:4 * _CR], wk_d.ap()[128:256, :])
            wv_sb = pers.tile([128, 2 * _C], bf16, name="wv_sb")
            nc.sync.dma_start(wv_sb[:, 0:_C], wv_d.ap()[0:128, :])
            nc.sync.dma_start(wv_sb[:, _C:2 * _C], wv_d.ap()[128:256, :])

            if has_bq:
                bq_sb = pers.tile([2 * _CR, 1], f32, name="bq_sb")
                nc.sync.dma_start(bq_sb, bq_d.ap())
            if has_bk:
                bk_sb = pers.tile([2 * _CR, 1], f32, name="bk_sb")
                nc.sync.dma_start(bk_sb, bk_d.ap())
            if has_bv:
                bv_sb = pers.tile([1, _C], bf16, name="bv_sb")
                nc.sync.dma_start(bv_sb, bv_d.ap())
                ones_row = pers.tile([1, 128], bf16, name="ones_row")
                nc.any.memset(ones_row, 1.0)

            xbf0 = pers.tile([128, _N], bf16, name="xbf0")
            xbf1 = pers.tile([128, _N], bf16, name="xbf1")
            xq0 = pers.tile([128, _HALF], f32, name="xq0")
            xq1 = pers.tile([128, _HALF], f32, name="xq1")
            xqbf0 = pers.tile([128, _HALF], bf16, name="xqbf0")
            xqbf1 = pers.tile([128, _HALF], bf16, name="xqbf1")
            # pair-packed K: row group t in {0,1}, k_pk[32t+d, p*128+jj]
            # = k[d, (2p+t)*128+jj]
            k_pk = pers.tile([64, NP * 128], bf16, name="k_pk")
            # q replicated in both row groups
            q_rep = pers.tile([64, _HALF], bf16, name="q_rep")
            vt_sb = pers.tile([128, NJ * VTW], bf16, name="vt_sb")

            # ---- phase 0: load x, cast, compute K / Q / V^T ----
            CH = 512
            with tc.tile_pool(name="prep_ps", space="PSUM", bufs=2) as pps:
                # xq first (unblocks Q, and residual later)
                for ch in range(_HALF // 1024):
                    sl = slice(ch * 1024, (ch + 1) * 1024)
                    nc.sync.dma_start(xq0[:, sl], xq_d.ap()[0:128, sl])
                    nc.vector.tensor_copy(xqbf0[:, sl], xq0[:, sl])
                    nc.sync.dma_start(xq1[:, sl], xq_d.ap()[128:256, sl])
                    nc.vector.tensor_copy(xqbf1[:, sl], xq1[:, sl])
                for ch in range(_N // CH):
                    sl = slice(ch * CH, (ch + 1) * CH)
                    xf0 = work.tile([128, CH], f32, name="xf0", tag="xf", bufs=4)
                    nc.sync.dma_start(xf0, x_d.ap()[0:128, sl])
                    nc.vector.tensor_copy(xbf0[:, sl], xf0)
                    xf1 = work.tile([128, CH], f32, name="xf1", tag="xf", bufs=4)
                    nc.sync.dma_start(xf1, x_d.ap()[128:256, sl])
                    nc.vector.tensor_copy(xbf1[:, sl], xf1)

                # Q: both row groups at once via doubled weights
                for b5 in range(_HALF // 512):
                    sl = slice(b5 * 512, (b5 + 1) * 512)
                    qp = pps.tile([64, 512], f32, name="qp", tag="prep", bufs=2)
                    nc.tensor.matmul(qp, wq_sb[:, 0:2 * _CR], xqbf0[:, sl],
                                     start=True, stop=False)
                    nc.tensor.matmul(qp, wq_sb[:, 2 * _CR:4 * _CR], xqbf1[:, sl],
                                     start=False, stop=True)
                    if has_bq:
                        nc.vector.tensor_scalar(q_rep[:, sl], qp, bq_sb, 0.0,
                                                ALU.add, ALU.max)
                    else:
                        nc.vector.tensor_scalar_max(q_rep[:, sl], qp, 0.0)
                # K: compute doubled, then pack pairs
                for b5 in range(_N // 512):
                    sl = slice(b5 * 512, (b5 + 1) * 512)
                    kp = pps.tile([64, 512], f32, name="kp", tag="prep", bufs=2)
                    nc.tensor.matmul(kp, wk_sb[:, 0:2 * _CR], xbf0[:, sl],
                                     start=True, stop=False)
                    nc.tensor.matmul(kp, wk_sb[:, 2 * _CR:4 * _CR], xbf1[:, sl],
                                     start=False, stop=True)
                    # 512-block b5 holds key blocks 4*b5..4*b5+3 = pairs
                    # (2*b5, t=0/1), (2*b5+1, t=0/1)
                    for c in range(4):
                        p, t = 2 * b5 + c // 2, c % 2
                        dst = k_pk[32 * t:32 * t + 32, p * 128:(p + 1) * 128]
                        src = kp[32 * t:32 * t + 32, c * 128:(c + 1) * 128]
                        if has_bk:
                            nc.vector.tensor_scalar(
                                dst, src, bk_sb[32 * t:32 * t + 32, :], 0.0,
                                ALU.add, ALU.max)
                        else:
                            nc.vector.tensor_scalar_max(dst, src, 0.0)
                # V^T: per key block [j=128, c=256], relu * gamma, ones col
                for jb in range(NJ):
                    jsl = slice(jb * 128, (jb + 1) * 128)
                    vp = pps.tile([128, _C], f32, name="vp", tag="prep", bufs=2)
                    nc.tensor.matmul(vp, xbf0[:, jsl], wv_sb[:, 0:_C], start=True,
                                     stop=not has_bv)
                    nc.tensor.matmul(vp, xbf1[:, jsl], wv_sb[:, _C:2 * _C],
                                     start=False, stop=not has_bv)
                    if has_bv:
                        nc.tensor.matmul(vp, ones_row, bv_sb, start=False, stop=True)
                    vsl = slice(jb * VTW, jb * VTW + _C)
                    nc.vector.tensor_scalar(vt_sb[:, vsl], vp, 0.0, g_sb,
                                            ALU.max, ALU.mult)
                    nc.any.memset(vt_sb[:, jb * VTW + _C:(jb + 1) * VTW], 1.0)

            # ---- phase 1: attention, software-pipelined over key pairs ----
            with tc.tile_pool(name="att_ps", space="PSUM", bufs=1) as aps:
                for i5 in range(NI5):
                    isl = slice(i5 * 512, (i5 + 1) * 512)
                    ots = [
                        aps.tile([128, VTW], f32, name=f"ot{s}", tag=f"ot{s}", bufs=1)
                        for s in range(4)
                    ]
                    e_tiles = [None] * NP

                    def qk_exp(p):
                        st = aps.tile([128, 1024], f32, name="st", tag="st", bufs=2)
                        for t in range(2):
                            nc.tensor.matmul(
                                st[:, t * 512:(t + 1) * 512],
                                k_pk[32 * t:32 * t + 32, p * 128:(p + 1) * 128],
                                q_rep[32 * t:32 * t + 32, isl],
                                start=True, stop=True,
                                tile_position=(32 * t, 0),
                            )
                        e = work.tile([128, 1024], bf16, name="e_sb", tag="e", bufs=3)
                        nc.scalar.activation(e, st, AF.Exp)
                        e_tiles[p] = e

                    def pv(p):
                        e = e_tiles[p]
                        for s in range(4):
                            for t in range(2):
                                jb = 2 * p + t
                                nc.tensor.matmul(
                                    ots[s],
                                    e[:, t * 512 + s * 128:t * 512 + (s + 1) * 128],
                                    vt_sb[:, jb * VTW:(jb + 1) * VTW],
                                    start=(jb == 0), stop=(jb == NJ - 1),
                                )
                        e_tiles[p] = None

                    for p in range(NP):
                        qk_exp(p)
                        if p > 0:
                            pv(p - 1)
                    pv(NP - 1)

                    for s in range(4):
                        i0 = i5 * 512 + s * 128
                        rz = work.tile([128, 1], f32, name="rz", tag="rz", bufs=2)
                        nc.vector.reciprocal(rz, ots[s][:, _C:_C + 1])
                        onrm = work.tile([128, _C], bf16, name="onrm", tag="onrm",
                                         bufs=2)
                        nc.vector.tensor_scalar_mul(onrm, ots[s][:, 0:_C], rz)
                        for chh in range(2):
                            tT = work.tile([128, 128], bf16, name="tT", tag="tT",
                                           bufs=3)
                            nc.sync.dma_start(
                                tT, onrm[:, chh * 128:(chh + 1) * 128],
                                transpose=True)
                            res = work.tile([128, 128], f32, name="res", tag="res",
                                            bufs=3)
                            xq_t = xq0 if chh == 0 else xq1
                            nc.vector.tensor_add(res, tT, xq_t[:, i0:i0 + 128])
                            nc.sync.dma_start(
                                out_d.ap()[chh * 128:(chh + 1) * 128, i0:i0 + 128],
                                res)

    nc.compile()
    return nc


def _get_nc(has_bq, has_bk, has_bv):
    key = (has_bq, has_bk, has_bv)
    if key not in _BUILD_CACHE:
        _BUILD_CACHE[key] = _build(*key)
    return _BUILD_CACHE[key]


def kernel(x, w_q, w_k, w_v,
           bn1_scale, bn1_bias, bn1_mean, bn1_var,
           bn2_scale, bn2_bias, bn2_mean, bn2_var,
           bn3_scale, bn3_bias, bn3_mean, bn3_var,
           gamma, _trace=False):
    from concourse.bass_utils import run_bass_kernel_spmd

    x = np.asarray(x, dtype=np.float32)
    gamma_f = float(np.asarray(gamma).reshape(-1)[0])
    bf = ml_dtypes.bfloat16

    def fold(w, s, b, m, v):
        a = np.asarray(s, np.float32) / np.sqrt(np.asarray(v, np.float32) + _EPS)
        return (np.asarray(w, np.float32) * a[:, None],
                np.asarray(b, np.float32) - np.asarray(m, np.float32) * a)

    wqf, bq = fold(w_q, bn1_scale, bn1_bias, bn1_mean, bn1_var)
    wkf, bk = fold(w_k, bn2_scale, bn2_bias, bn2_mean, bn2_var)
    wvf, bv = fold(w_v, bn3_scale, bn3_bias, bn3_mean, bn3_var)
    has_bq = bool(np.any(bq != 0.0))
    has_bk = bool(np.any(bk != 0.0))
    has_bv = bool(np.any(bv != 0.0))

    nc = _get_nc(has_bq, has_bk, has_bv)

    wqt2 = np.tile(np.ascontiguousarray(wqf.T), (1, 2)).astype(bf)  # [c_in, 2cr]
    wkt2 = np.tile(np.ascontiguousarray(wkf.T), (1, 2)).astype(bf)
    wvt = np.ascontiguousarray(wvf.T).astype(bf)                    # [c_in, c_out]
    gvec = np.full((128, 1), gamma_f, dtype=np.float32)

    in_maps = []
    for core in range(_NCORES):
        b, h = divmod(core, 2)
        m = {
            "x": np.ascontiguousarray(x[b]),
            "xq": np.ascontiguousarray(x[b][:, h * _HALF:(h + 1) * _HALF]),
            "wqt2": wqt2, "wkt2": wkt2, "wvt": wvt, "gvec": gvec,
        }
        if has_bq:
            m["bq2"] = np.ascontiguousarray(np.tile(bq, 2).reshape(2 * _CR, 1))
        if has_bk:
            m["bk2"] = np.ascontiguousarray(np.tile(bk, 2).reshape(2 * _CR, 1))
        if has_bv:
            m["bv"] = np.ascontiguousarray(bv.reshape(1, _C)).astype(bf)
        in_maps.append(m)

    res = run_bass_kernel_spmd(nc, in_maps, core_ids=list(range(_NCORES)),
                               trace=_trace)

    out = np.empty((_B, _C, _N), dtype=np.float32)
    for core in range(_NCORES):
        b, h = divmod(core, 2)
        out[b, :, h * _HALF:(h + 1) * _HALF] = res.results[core]["out"]
    if _trace:
        kernel.last_results = res
    return out


# revision 8
# speedup vs baseline: 1.3335x; 1.1443x over previous
"""Criss-cross attention (width=1) Trainium2 Bass kernel.

Math note: for width=1 the criss-cross module collapses to plain unmasked
softmax attention.  The diagonal of energy_H is masked to -inf, but the
"width" logit energy_W[i] equals that same diagonal value q_i.k_i, and it is
re-appended as the (n+1)-th softmax entry.  So per query i the softmax runs
over exactly {q_i.k_j : j=0..n-1}, and

    out = gamma * (V @ softmax_j(Q^T K)) + x
    Q = relu(bn1(w_q x)),  K = relu(bn2(w_k x)),  V = relu(bn3(w_v x))

Sharding: 8 cores = (4 batches) x (2 query halves).  Each core computes
K, V for all 4096 keys of its batch and attention output for its 2048
queries.  Zero cross-core communication.

Per-core structure (matmuls bf16-in / f32-psum):
  S^T[j,i] = K^T Q, computed pair-packed: two K row-groups (K=32
  contraction each) run concurrently via tile_position, filling a
  [128, 1024] PSUM pair tile (two key blocks x 512 queries).
  E = exp(S^T) -> bf16 (no max subtraction needed: logits <= ~30).
  O^T[i, 0:256] accumulates E_blk^T @ [V^T | 1]; the ones column makes
  O^T[i, 256] = Z_i.  Epilogue: per-partition reciprocal, scale (gamma
  pre-folded into V^T), DMA-transpose back to [c, i], residual add.
  The j-loop is software-pipelined: PE runs PV of pair p-1 while ACT
  exponentiates pair p, keeping the tensor engine dense (HAM-warm).
"""

import os
import numpy as np
import ml_dtypes

_B, _C, _N, _CR = 4, 256, 4096, 32
_NCORES = 8
_HALF = _N // 2  # queries per core
_EPS = 1e-5

_BUILD_CACHE: dict = {}


def _build(has_bq: bool, has_bk: bool, has_bv: bool):
    import concourse.mybir as mybir
    import concourse.tile as tile
    from concourse import bacc

    f32 = mybir.dt.float32
    bf16 = mybir.dt.bfloat16
    AF = mybir.ActivationFunctionType
    ALU = mybir.AluOpType

    nc = bacc.Bacc("TRN2", target_bir_lowering=False, debug=False)

    x_d = nc.dram_tensor("x", [_C, _N], f32, kind="ExternalInput")
    xq_d = nc.dram_tensor("xq", [_C, _HALF], f32, kind="ExternalInput")
    wq_d = nc.dram_tensor("wqt2", [_C, 2 * _CR], bf16, kind="ExternalInput")
    wk_d = nc.dram_tensor("wkt2", [_C, 2 * _CR], bf16, kind="ExternalInput")
    wv_d = nc.dram_tensor("wvt", [_C, _C], bf16, kind="ExternalInput")
    g_d = nc.dram_tensor("gvec", [128, 1], f32, kind="ExternalInput")
    bq_d = nc.dram_tensor("bq2", [2 * _CR, 1], f32, kind="ExternalInput") if has_bq else None
    bk_d = nc.dram_tensor("bk2", [2 * _CR, 1], f32, kind="ExternalInput") if has_bk else None
    bv_d = nc.dram_tensor("bv", [1, _C], bf16, kind="ExternalInput") if has_bv else None
    out_d = nc.dram_tensor("out", [_C, _HALF], f32, kind="ExternalOutput")

    NJ = _N // 128        # 32 key blocks
    NP = NJ // 2          # 16 key pairs
    NI5 = _HALF // 512    # 4 query super-blocks
    VTW = _C + 1          # 257: V^T columns + ones column for Z

    with tile.TileContext(nc) as tc:
        with tc.tile_pool(name="persist", bufs=1) as pers, \
             tc.tile_pool(name="work", bufs=2) as work:
            # ---- persistent SBUF tensors ----
            g_sb = pers.tile([128, 1], f32, name="g_sb")
            nc.sync.dma_start(g_sb, g_d.ap())

            wq_sb = pers.tile([128, 4 * _CR], bf16, name="wq_sb")
            nc.sync.dma_start(wq_sb[:, 0:2 * _CR], wq_d.ap()[0:128, :])
            nc.sync.dma_start(wq_sb[:, 2 * _CR:4 * _CR], wq_d.ap()[128:256, :])
            wk_sb = pers.tile([128, 4 * _CR], bf16, name="wk_sb")
            nc.sync.dma_start(wk_sb[:, 0:2 * _CR], wk_d.ap()[0:128, :])
            nc.sync.dma_start(wk_sb[:, 2 * _CR:4 * _CR], wk_d.ap()[128:256, :])
            wv_sb = pers.tile([128, 2 * _C], bf16, name="wv_sb")
            nc.sync.dma_start(wv_sb[:, 0:_C], wv_d.ap()[0:128, :])
            nc.sync.dma_start(wv_sb[:, _C:2 * _C], wv_d.ap()[128:256, :])

            if has_bq:
                bq_sb = pers.tile([2 * _CR, 1], f32, name="bq_sb")
                nc.sync.dma_start(bq_sb, bq_d.ap())
            if has_bk:
                bk_sb = pers.tile([2 * _CR, 1], f32, name="bk_sb")
                nc.sync.dma_start(bk_sb, bk_d.ap())
            if has_bv:
                bv_sb = pers.tile([1, _C], bf16, name="bv_sb")
                nc.sync.dma_start(bv_sb, bv_d.ap())
                ones_row = pers.tile([1, 128], bf16, name="ones_row")
                nc.any.memset(ones_row, 1.0)

            xbf0 = pers.tile([128, _N], bf16, name="xbf0")
            xbf1 = pers.tile([128, _N], bf16, name="xbf1")
            xq0 = pers.tile([128, _HALF], f32, name="xq0")
            xq1 = pers.tile([128, _HALF], f32, name="xq1")
            xqbf0 = pers.tile([128, _HALF], bf16, name="xqbf0")
            xqbf1 = pers.tile([128, _HALF], bf16, name="xqbf1")
            # pair-packed K: row group t in {0,1}, k_pk[32t+d, p*128+jj]
            # = k[d, (2p+t)*128+jj]
            k_pk = pers.tile([64, NP * 128], bf16, name="k_pk")
            # q replicated in both row groups
            q_rep = pers.tile([64, _HALF], bf16, name="q_rep")
            vt_sb = pers.tile([128, NJ * VTW], bf16, name="vt_sb")

            # ---- build helpers ----
            def load_xq_and_q(pps):
                # xq + Q (cheap, unblocks attention pair 0 and residuals)
                for ch in range(_HALF // 1024):
                    sl = slice(ch * 1024, (ch + 1) * 1024)
                    nc.sync.dma_start(xq0[:, sl], xq_d.ap()[0:128, sl])
                    nc.vector.tensor_copy(xqbf0[:, sl], xq0[:, sl])
                    nc.gpsimd.dma_start(xq1[:, sl], xq_d.ap()[128:256, sl])
                    nc.vector.tensor_copy(xqbf1[:, sl], xq1[:, sl])
                for b5 in range(_HALF // 512):
                    sl = slice(b5 * 512, (b5 + 1) * 512)
                    qp = pps.tile([64, 512], f32, name="qp", tag="st", bufs=2)
                    nc.tensor.matmul(qp, wq_sb[:, 0:2 * _CR], xqbf0[:, sl],
                                     start=True, stop=False)
                    nc.tensor.matmul(qp, wq_sb[:, 2 * _CR:4 * _CR], xqbf1[:, sl],
                                     start=False, stop=True)
                    if has_bq:
                        nc.vector.tensor_scalar(q_rep[:, sl], qp, bq_sb, 0.0,
                                                ALU.add, ALU.max)
                    else:
                        nc.vector.tensor_scalar_max(q_rep[:, sl], qp, 0.0)

            def prep_chunk(pps, b5):
                """Load/cast x columns [b5*512, (b5+1)*512), compute K pairs
                2*b5, 2*b5+1 and V^T blocks 4*b5..4*b5+3."""
                sl = slice(b5 * 512, (b5 + 1) * 512)
                xf0 = work.tile([128, 512], f32, name="xf0", tag="xf", bufs=4)
                nc.sync.dma_start(xf0, x_d.ap()[0:128, sl])
                nc.vector.tensor_copy(xbf0[:, sl], xf0)
                xf1 = work.tile([128, 512], f32, name="xf1", tag="xf", bufs=4)
                nc.gpsimd.dma_start(xf1, x_d.ap()[128:256, sl])
                nc.vector.tensor_copy(xbf1[:, sl], xf1)

                kp = pps.tile([64, 512], f32, name="kp", tag="st", bufs=2)
                nc.tensor.matmul(kp, wk_sb[:, 0:2 * _CR], xbf0[:, sl],
                                 start=True, stop=False)
                nc.tensor.matmul(kp, wk_sb[:, 2 * _CR:4 * _CR], xbf1[:, sl],
                                 start=False, stop=True)
                for c in range(4):
                    p, t = 2 * b5 + c // 2, c % 2
                    dst = k_pk[32 * t:32 * t + 32, p * 128:(p + 1) * 128]
                    src = kp[32 * t:32 * t + 32, c * 128:(c + 1) * 128]
                    if has_bk:
                        nc.vector.tensor_scalar(
                            dst, src, bk_sb[32 * t:32 * t + 32, :], 0.0,
                            ALU.add, ALU.max)
                    else:
                        nc.vector.tensor_scalar_max(dst, src, 0.0)
                for jb in range(4 * b5, 4 * b5 + 4):
                    jsl = slice(jb * 128, (jb + 1) * 128)
                    vp = pps.tile([128, _C], f32, name="vp", tag="st", bufs=2)
                    nc.tensor.matmul(vp, xbf0[:, jsl], wv_sb[:, 0:_C], start=True,
                                     stop=not has_bv)
                    nc.tensor.matmul(vp, xbf1[:, jsl], wv_sb[:, _C:2 * _C],
                                     start=False, stop=not has_bv)
                    if has_bv:
                        nc.tensor.matmul(vp, ones_row, bv_sb, start=False, stop=True)
                    vsl = slice(jb * VTW, jb * VTW + _C)
                    nc.vector.tensor_scalar(vt_sb[:, vsl], vp, 0.0, g_sb,
                                            ALU.max, ALU.mult)
                    nc.any.memset(vt_sb[:, jb * VTW + _C:(jb + 1) * VTW], 1.0)

            # ---- attention (software-pipelined over key pairs); prep for
            # x-chunk b5 is interleaved into the first query block so the
            # tensor engine stays dense from the start ----
            with tc.tile_pool(name="att_ps", space="PSUM", bufs=1) as aps:
                pps = aps  # prep PSUM tiles share the "st" tag slots
                load_xq_and_q(pps)

                for i5 in range(NI5):
                    isl = slice(i5 * 512, (i5 + 1) * 512)
                    ots = [
                        aps.tile([128, VTW], f32, name=f"ot{s}", tag=f"ot{s}", bufs=1)
                        for s in range(4)
                    ]
                    e_tiles = [None] * NP

                    def qk_exp(p):
                        st = aps.tile([128, 1024], f32, name="st", tag="st", bufs=2)
                        for t in range(2):
                            nc.tensor.matmul(
                                st[:, t * 512:(t + 1) * 512],
                                k_pk[32 * t:32 * t + 32, p * 128:(p + 1) * 128],
                                q_rep[32 * t:32 * t + 32, isl],
                                start=True, stop=True,
                                tile_position=(32 * t, 0),
                            )
                        e = work.tile([128, 1024], bf16, name="e_sb", tag="e", bufs=3)
                        nc.scalar.activation(e, st, AF.Exp)
                        e_tiles[p] = e

                    def pv(p):
                        e = e_tiles[p]
                        for s in range(4):
                            for t in range(2):
                                jb = 2 * p + t
                                nc.tensor.matmul(
                                    ots[s],
                                    e[:, t * 512 + s * 128:t * 512 + (s + 1) * 128],
                                    vt_sb[:, jb * VTW:(jb + 1) * VTW],
                                    start=(jb == 0), stop=(jb == NJ - 1),
                                )
                        e_tiles[p] = None

                    for p in range(NP):
                        if i5 == 0 and p % 2 == 0:
                            prep_chunk(pps, p // 2)
                        qk_exp(p)
                        if p > 0:
                            pv(p - 1)
                    pv(NP - 1)

                    # epilogue: free the ot accumulator banks as fast as
                    # possible (recip+scale first), then transpose/add/store
                    rzs, onrms = [], []
                    for s in range(4):
                        rz = work.tile([128, 1], f32, name="rz", tag=f"rz{s}",
                                       bufs=2)
                        nc.vector.reciprocal(rz, ots[s][:, _C:_C + 1])
                        onrm = work.tile([128, _C], bf16, name="onrm",
                                         tag=f"onrm{s}", bufs=2)
                        nc.vector.tensor_scalar_mul(onrm, ots[s][:, 0:_C], rz)
                        rzs.append(rz)
                        onrms.append(onrm)
                    for s in range(4):
                        i0 = i5 * 512 + s * 128
                        for chh in range(2):
                            tT = work.tile([128, 128], bf16, name="tT", tag="tT",
                                           bufs=4)
                            nc.sync.dma_start(
                                tT, onrms[s][:, chh * 128:(chh + 1) * 128],
                                transpose=True)
                            res = work.tile([128, 128], f32, name="res", tag="res",
                                            bufs=4)
                            xq_t = xq0 if chh == 0 else xq1
                            nc.vector.tensor_add(res, tT, xq_t[:, i0:i0 + 128])
                            nc.gpsimd.dma_start(
                                out_d.ap()[chh * 128:(chh + 1) * 128, i0:i0 + 128],
                                res)

    nc.compile()
    return nc


def _get_nc(has_bq, has_bk, has_bv):
    key = (has_bq, has_bk, has_bv)
    if key not in _BUILD_CACHE:
        _BUILD_CACHE[key] = _build(*key)
    return _BUILD_CACHE[key]


def kernel(x, w_q, w_k, w_v,
           bn1_scale, bn1_bias, bn1_mean, bn1_var,
           bn2_scale, bn2_bias, bn2_mean, bn2_var,
           bn3_scale, bn3_bias, bn3_mean, bn3_var,
           gamma, _trace=False):
    from concourse.bass_utils import run_bass_kernel_spmd

    x = np.asarray(x, dtype=np.float32)
    gamma_f = float(np.asarray(gamma).reshape(-1)[0])
    bf = ml_dtypes.bfloat16

    def fold(w, s, b, m, v):
        a = np.asarray(s, np.float32) / np.sqrt(np.asarray(v, np.float32) + _EPS)
        return (np.asarray(w, np.float32) * a[:, None],
                np.asarray(b, np.float32) - np.asarray(m, np.float32) * a)

    wqf, bq = fold(w_q, bn1_scale, bn1_bias, bn1_mean, bn1_var)
    wkf, bk = fold(w_k, bn2_scale, bn2_bias, bn2_mean, bn2_var)
    wvf, bv = fold(w_v, bn3_scale, bn3_bias, bn3_mean, bn3_var)
    has_bq = bool(np.any(bq != 0.0))
    has_bk = bool(np.any(bk != 0.0))
    has_bv = bool(np.any(bv != 0.0))

    nc = _get_nc(has_bq, has_bk, has_bv)

    wqt2 = np.tile(np.ascontiguousarray(wqf.T), (1, 2)).astype(bf)  # [c_in, 2cr]
    wkt2 = np.tile(np.ascontiguousarray(wkf.T), (1, 2)).astype(bf)
    wvt = np.ascontiguousarray(wvf.T).astype(bf)                    # [c_in, c_out]
    gvec = np.full((128, 1), gamma_f, dtype=np.float32)

    in_maps = []
    for core in range(_NCORES):
        b, h = divmod(core, 2)
        m = {
            "x": np.ascontiguousarray(x[b]),
            "xq": np.ascontiguousarray(x[b][:, h * _HALF:(h + 1) * _HALF]),
            "wqt2": wqt2, "wkt2": wkt2, "wvt": wvt, "gvec": gvec,
        }
        if has_bq:
            m["bq2"] = np.ascontiguousarray(np.tile(bq, 2).reshape(2 * _CR, 1))
        if has_bk:
            m["bk2"] = np.ascontiguousarray(np.tile(bk, 2).reshape(2 * _CR, 1))
        if has_bv:
            m["bv"] = np.ascontiguousarray(bv.reshape(1, _C)).astype(bf)
        in_maps.append(m)

    res = run_bass_kernel_spmd(nc, in_maps, core_ids=list(range(_NCORES)),
                               trace=_trace)

    out = np.empty((_B, _C, _N), dtype=np.float32)
    for core in range(_NCORES):
        b, h = divmod(core, 2)
        out[b, :, h * _HALF:(h + 1) * _HALF] = res.results[core]["out"]
    if _trace:
        kernel.last_results = res
    return out


# revision 11
# speedup vs baseline: 1.4481x; 1.0859x over previous
"""Criss-cross attention (width=1) Trainium2 Bass kernel.

Math note: for width=1 the criss-cross module collapses to plain unmasked
softmax attention.  The diagonal of energy_H is masked to -inf, but the
"width" logit energy_W[i] equals that same diagonal value q_i.k_i, and it is
re-appended as the (n+1)-th softmax entry.  So per query i the softmax runs
over exactly {q_i.k_j : j=0..n-1}, and

    out = gamma * (V @ softmax_j(Q^T K)) + x
    Q = relu(bn1(w_q x)),  K = relu(bn2(w_k x)),  V = relu(bn3(w_v x))

Sharding: 8 cores = (4 batches) x (2 query halves).  Each core computes
K, V for all 4096 keys of its batch and attention output for its 2048
queries.  Zero cross-core communication.

Per-core structure (matmuls bf16-in / f32-psum):
  S^T[j,i] = K^T Q, computed pair-packed: two K row-groups (K=32
  contraction each) run concurrently via tile_position, filling a
  [128, 1024] PSUM pair tile (two key blocks x 512 queries).
  E = exp(S^T) -> bf16 (no max subtraction needed: logits <= ~30).
  O^T[i, 0:256] accumulates E_blk^T @ [V^T | 1]; the ones column makes
  O^T[i, 256] = Z_i.  Epilogue: per-partition reciprocal, scale (gamma
  pre-folded into V^T), DMA-transpose back to [c, i], residual add.
  The j-loop is software-pipelined: PE runs PV of pair p-1 while ACT
  exponentiates pair p, keeping the tensor engine dense (HAM-warm).
"""

import os
import numpy as np
import ml_dtypes

_B, _C, _N, _CR = 4, 256, 4096, 32
_NCORES = 8
_HALF = _N // 2  # queries per core
_EPS = 1e-5

_BUILD_CACHE: dict = {}


def _build(has_bq: bool, has_bk: bool, has_bv: bool):
    import concourse.mybir as mybir
    import concourse.tile as tile
    from concourse import bacc

    f32 = mybir.dt.float32
    bf16 = mybir.dt.bfloat16
    AF = mybir.ActivationFunctionType
    ALU = mybir.AluOpType

    nc = bacc.Bacc("TRN2", target_bir_lowering=False, debug=False)

    x_d = nc.dram_tensor("x", [_C, _N], f32, kind="ExternalInput")
    xq_d = nc.dram_tensor("xq", [_C, _HALF], f32, kind="ExternalInput")
    wq_d = nc.dram_tensor("wqt2", [_C, 2 * _CR], bf16, kind="ExternalInput")
    wk_d = nc.dram_tensor("wkt2", [_C, 2 * _CR], bf16, kind="ExternalInput")
    wv_d = nc.dram_tensor("wvt", [_C, _C], bf16, kind="ExternalInput")
    g_d = nc.dram_tensor("gvec", [128, 1], f32, kind="ExternalInput")
    bq_d = nc.dram_tensor("bq2", [2 * _CR, 1], f32, kind="ExternalInput") if has_bq else None
    bk_d = nc.dram_tensor("bk2", [2 * _CR, 1], f32, kind="ExternalInput") if has_bk else None
    bv_d = nc.dram_tensor("bv", [1, _C], bf16, kind="ExternalInput") if has_bv else None
    out_d = nc.dram_tensor("out", [_C, _HALF], f32, kind="ExternalOutput")

    NJ = _N // 128        # 32 key blocks
    NP = NJ // 2          # 16 key pairs
    NI5 = _HALF // 512    # 4 query super-blocks
    VTW = _C + 1          # 257: V^T columns + ones column for Z

    with tile.TileContext(nc) as tc:
        with tc.tile_pool(name="persist", bufs=1) as pers, \
             tc.tile_pool(name="work", bufs=2) as work:
            # ---- persistent SBUF tensors ----
            g_sb = pers.tile([128, 1], f32, name="g_sb")
            nc.sync.dma_start(g_sb, g_d.ap())

            ident = pers.tile([128, 128], bf16, name="ident")
            from concourse.masks import make_identity
            make_identity(nc, ident)

            wq_sb = pers.tile([128, 4 * _CR], bf16, name="wq_sb")
            nc.sync.dma_start(wq_sb[:, 0:2 * _CR], wq_d.ap()[0:128, :])
            nc.sync.dma_start(wq_sb[:, 2 * _CR:4 * _CR], wq_d.ap()[128:256, :])
            wk_sb = pers.tile([128, 4 * _CR], bf16, name="wk_sb")
            nc.sync.dma_start(wk_sb[:, 0:2 * _CR], wk_d.ap()[0:128, :])
            nc.sync.dma_start(wk_sb[:, 2 * _CR:4 * _CR], wk_d.ap()[128:256, :])
            wv_sb = pers.tile([128, 2 * _C], bf16, name="wv_sb")
            nc.sync.dma_start(wv_sb[:, 0:_C], wv_d.ap()[0:128, :])
            nc.sync.dma_start(wv_sb[:, _C:2 * _C], wv_d.ap()[128:256, :])

            if has_bq:
                bq_sb = pers.tile([2 * _CR, 1], f32, name="bq_sb")
                nc.sync.dma_start(bq_sb, bq_d.ap())
            if has_bk:
                bk_sb = pers.tile([2 * _CR, 1], f32, name="bk_sb")
                nc.sync.dma_start(bk_sb, bk_d.ap())
            if has_bv:
                bv_sb = pers.tile([1, _C], bf16, name="bv_sb")
                nc.sync.dma_start(bv_sb, bv_d.ap())
                ones_row = pers.tile([1, 128], bf16, name="ones_row")
                nc.any.memset(ones_row, 1.0)

            xbf0 = pers.tile([128, _N], bf16, name="xbf0")
            xbf1 = pers.tile([128, _N], bf16, name="xbf1")
            xq0 = pers.tile([128, _HALF], f32, name="xq0")
            xq1 = pers.tile([128, _HALF], f32, name="xq1")
            xqbf0 = pers.tile([128, _HALF], bf16, name="xqbf0")
            xqbf1 = pers.tile([128, _HALF], bf16, name="xqbf1")
            # pair-packed K: row group t in {0,1}, k_pk[32t+d, p*128+jj]
            # = k[d, (2p+t)*128+jj]
            k_pk = pers.tile([64, NP * 128], bf16, name="k_pk")
            # q replicated in both row groups
            q_rep = pers.tile([64, _HALF], bf16, name="q_rep")
            vt_sb = pers.tile([128, NJ * VTW], bf16, name="vt_sb")

            # ---- build helpers ----
            def load_xq_and_q(pps):
                # xq + Q (cheap, unblocks attention pair 0 and residuals)
                for ch in range(_HALF // 1024):
                    sl = slice(ch * 1024, (ch + 1) * 1024)
                    nc.sync.dma_start(xq0[:, sl], xq_d.ap()[0:128, sl])
                    nc.vector.tensor_copy(xqbf0[:, sl], xq0[:, sl])
                    nc.gpsimd.dma_start(xq1[:, sl], xq_d.ap()[128:256, sl])
                    nc.vector.tensor_copy(xqbf1[:, sl], xq1[:, sl])
                for b5 in range(_HALF // 512):
                    sl = slice(b5 * 512, (b5 + 1) * 512)
                    qp = pps.tile([64, 512], f32, name="qp", tag="st", bufs=2)
                    nc.tensor.matmul(qp, wq_sb[:, 0:2 * _CR], xqbf0[:, sl],
                                     start=True, stop=False)
                    nc.tensor.matmul(qp, wq_sb[:, 2 * _CR:4 * _CR], xqbf1[:, sl],
                                     start=False, stop=True)
                    if has_bq:
                        nc.vector.tensor_scalar(q_rep[:, sl], qp, bq_sb, 0.0,
                                                ALU.add, ALU.max)
                    else:
                        nc.vector.tensor_scalar_max(q_rep[:, sl], qp, 0.0)

            def prep_chunk(pps, b5):
                """Load/cast x columns [b5*512, (b5+1)*512), compute K pairs
                2*b5, 2*b5+1 and V^T blocks 4*b5..4*b5+3."""
                sl = slice(b5 * 512, (b5 + 1) * 512)
                xf0 = work.tile([128, 512], f32, name="xf0", tag="xf", bufs=4)
                nc.sync.dma_start(xf0, x_d.ap()[0:128, sl])
                nc.vector.tensor_copy(xbf0[:, sl], xf0)
                xf1 = work.tile([128, 512], f32, name="xf1", tag="xf", bufs=4)
                nc.gpsimd.dma_start(xf1, x_d.ap()[128:256, sl])
                nc.vector.tensor_copy(xbf1[:, sl], xf1)

                kp = pps.tile([64, 512], f32, name="kp", tag="st", bufs=2)
                nc.tensor.matmul(kp, wk_sb[:, 0:2 * _CR], xbf0[:, sl],
                                 start=True, stop=False)
                nc.tensor.matmul(kp, wk_sb[:, 2 * _CR:4 * _CR], xbf1[:, sl],
                                 start=False, stop=True)
                for c in range(4):
                    p, t = 2 * b5 + c // 2, c % 2
                    dst = k_pk[32 * t:32 * t + 32, p * 128:(p + 1) * 128]
                    src = kp[32 * t:32 * t + 32, c * 128:(c + 1) * 128]
                    if has_bk:
                        nc.vector.tensor_scalar(
                            dst, src, bk_sb[32 * t:32 * t + 32, :], 0.0,
                            ALU.add, ALU.max)
                    else:
                        nc.vector.tensor_scalar_max(dst, src, 0.0)
                for jb in range(4 * b5, 4 * b5 + 4):
                    jsl = slice(jb * 128, (jb + 1) * 128)
                    vp = pps.tile([128, _C], f32, name="vp", tag="st", bufs=2)
                    nc.tensor.matmul(vp, xbf0[:, jsl], wv_sb[:, 0:_C], start=True,
                                     stop=not has_bv)
                    nc.tensor.matmul(vp, xbf1[:, jsl], wv_sb[:, _C:2 * _C],
                                     start=False, stop=not has_bv)
                    if has_bv:
                        nc.tensor.matmul(vp, ones_row, bv_sb, start=False, stop=True)
                    vsl = slice(jb * VTW, jb * VTW + _C)
                    nc.vector.tensor_scalar(vt_sb[:, vsl], vp, 0.0, g_sb,
                                            ALU.max, ALU.mult)
                    nc.any.memset(vt_sb[:, jb * VTW + _C:(jb + 1) * VTW], 1.0)

            # ---- attention (software-pipelined over key pairs); prep for
            # x-chunk b5 is interleaved into the first query block so the
            # tensor engine stays dense from the start ----
            with tc.tile_pool(name="att_ps", space="PSUM", bufs=1) as aps:
                pps = aps  # prep PSUM tiles share the "st" tag slots
                load_xq_and_q(pps)

                for i5 in range(NI5):
                    isl = slice(i5 * 512, (i5 + 1) * 512)
                    ots = [
                        aps.tile([128, VTW], f32, name=f"ot{s}", tag=f"ot{s}", bufs=1)
                        for s in range(4)
                    ]
                    e_tiles = [None] * NP

                    def qk_exp(p):
                        st = aps.tile([128, 1024], f32, name="st", tag="st", bufs=2)
                        for t in range(2):
                            nc.tensor.matmul(
                                st[:, t * 512:(t + 1) * 512],
                                k_pk[32 * t:32 * t + 32, p * 128:(p + 1) * 128],
                                q_rep[32 * t:32 * t + 32, isl],
                                start=True, stop=True,
                                tile_position=(32 * t, 0),
                            )
                        e = work.tile([128, 1024], bf16, name="e_sb", tag="e", bufs=3)
                        nc.scalar.activation(e, st, AF.Exp)
                        e_tiles[p] = e

                    def pv(p):
                        e = e_tiles[p]
                        for s in range(4):
                            for t in range(2):
                                jb = 2 * p + t
                                nc.tensor.matmul(
                                    ots[s],
                                    e[:, t * 512 + s * 128:t * 512 + (s + 1) * 128],
                                    vt_sb[:, jb * VTW:(jb + 1) * VTW],
                                    start=(jb == 0), stop=(jb == NJ - 1),
                                )
                        e_tiles[p] = None

                    for p in range(NP):
                        if i5 == 0 and p % 2 == 0:
                            prep_chunk(pps, p // 2)
                        qk_exp(p)
                        if p > 0:
                            pv(p - 1)
                    pv(NP - 1)

                    # epilogue: free the ot accumulator banks as fast as
                    # possible (recip+scale first), then transpose/add/store
                    rzs, onrms = [], []
                    for s in range(4):
                        rz = work.tile([128, 1], f32, name="rz", tag=f"rz{s}",
                                       bufs=2)
                        nc.vector.reciprocal(rz, ots[s][:, _C:_C + 1])
                        onrm = work.tile([128, _C], bf16, name="onrm",
                                         tag=f"onrm{s}", bufs=2)
                        nc.vector.tensor_scalar_mul(onrm, ots[s][:, 0:_C], rz)
                        rzs.append(rz)
                        onrms.append(onrm)
                    last = i5 == NI5 - 1
                    for s in range(4):
                        i0 = i5 * 512 + s * 128
                        for chh in range(2):
                            xq_t = xq0 if chh == 0 else xq1
                            res = work.tile([128, 128], f32, name="res", tag="res",
                                            bufs=4)
                            if last:
                                # PE is idle now and the st PSUM slots are
                                # free; PE transpose beats the ~1.2us
                                # serialized DMA transposes for the tail.
                                tp = aps.tile([128, 128], bf16, name="tp",
                                              tag="st", bufs=2)
                                nc.tensor.transpose(
                                    tp, onrms[s][:, chh * 128:(chh + 1) * 128],
                                    ident)
                                nc.vector.tensor_add(res, tp,
                                                     xq_t[:, i0:i0 + 128])
                            else:
                                tT = work.tile([128, 128], bf16, name="tT",
                                               tag="tT", bufs=4)
                                nc.sync.dma_start(
                                    tT, onrms[s][:, chh * 128:(chh + 1) * 128],
                                    transpose=True)
                                nc.vector.tensor_add(res, tT,
                                                     xq_t[:, i0:i0 + 128])
                            dma_eng = nc.sync if last else nc.gpsimd
                            dma_eng.dma_start(
                                out_d.ap()[chh * 128:(chh + 1) * 128, i0:i0 + 128],
                                res)

    nc.compile()
    return nc


def _get_nc(has_bq, has_bk, has_bv):
    key = (has_bq, has_bk, has_bv)
    if key not in _BUILD_CACHE:
        _BUILD_CACHE[key] = _build(*key)
    return _BUILD_CACHE[key]


def kernel(x, w_q, w_k, w_v,
           bn1_scale, bn1_bias, bn1_mean, bn1_var,
           bn2_scale, bn2_bias, bn2_mean, bn2_var,
           bn3_scale, bn3_bias, bn3_mean, bn3_var,
           gamma, _trace=False):
    from concourse.bass_utils import run_bass_kernel_spmd

    x = np.asarray(x, dtype=np.float32)
    gamma_f = float(np.asarray(gamma).reshape(-1)[0])
    bf = ml_dtypes.bfloat16

    def fold(w, s, b, m, v):
        a = np.asarray(s, np.float32) / np.sqrt(np.asarray(v, np.float32) + _EPS)
        return (np.asarray(w, np.float32) * a[:, None],
                np.asarray(b, np.float32) - np.asarray(m, np.float32) * a)

    wqf, bq = fold(w_q, bn1_scale, bn1_bias, bn1_mean, bn1_var)
    wkf, bk = fold(w_k, bn2_scale, bn2_bias, bn2_mean, bn2_var)
    wvf, bv = fold(w_v, bn3_scale, bn3_bias, bn3_mean, bn3_var)
    has_bq = bool(np.any(bq != 0.0))
    has_bk = bool(np.any(bk != 0.0))
    has_bv = bool(np.any(bv != 0.0))

    nc = _get_nc(has_bq, has_bk, has_bv)

    wqt2 = np.tile(np.ascontiguousarray(wqf.T), (1, 2)).astype(bf)  # [c_in, 2cr]
    wkt2 = np.tile(np.ascontiguousarray(wkf.T), (1, 2)).astype(bf)
    wvt = np.ascontiguousarray(wvf.T).astype(bf)                    # [c_in, c_out]
    gvec = np.full((128, 1), gamma_f, dtype=np.float32)

    in_maps = []
    for core in range(_NCORES):
        b, h = divmod(core, 2)
        m = {
            "x": np.ascontiguousarray(x[b]),
            "xq": np.ascontiguousarray(x[b][:, h * _HALF:(h + 1) * _HALF]),
            "wqt2": wqt2, "wkt2": wkt2, "wvt": wvt, "gvec": gvec,
        }
        if has_bq:
            m["bq2"] = np.ascontiguousarray(np.tile(bq, 2).reshape(2 * _CR, 1))
        if has_bk:
            m["bk2"] = np.ascontiguousarray(np.tile(bk, 2).reshape(2 * _CR, 1))
        if has_bv:
            m["bv"] = np.ascontiguousarray(bv.reshape(1, _C)).astype(bf)
        in_maps.append(m)

    res = run_bass_kernel_spmd(nc, in_maps, core_ids=list(range(_NCORES)),
                               trace=_trace)

    out = np.empty((_B, _C, _N), dtype=np.float32)
    for core in range(_NCORES):
        b, h = divmod(core, 2)
        out[b, :, h * _HALF:(h + 1) * _HALF] = res.results[core]["out"]
    if _trace:
        kernel.last_results = res
    return out
